# revision 13
# baseline (speedup 1.0000x reference)
"""Trainium2 Bass kernel for the contrastive loss problem — v3.

v3 over v2:
 * Asymmetric padding: the normal side pads to RWP = ceil(max_nn/128)*128
   rows/cols, the anomaly side to RWN = ceil(max_na/128)*128 cols, chosen at
   runtime from the actual inputs (compile is seconds and cached per shape).
 * Input DMAs spread over 4 queues; unit sequence starts and ends with pos
   units; Ln in 2 groups.
 * D-unit (schraudolph) count picked to balance ACT vs DVE busy time.
"""

import numpy as np

UNIT = 2048        # PSUM staging tile width (4 banks); ping-pong 2 tiles = 8 banks
N_CORES = 8
EPS = 1e-6

SCHR_A = 184.66448852539062   # 128/ln2 rounded to fp32
SCHR_B = 16256.0              # 128*127
C_SCHR = 0.02003645           # E[ln(1+e_hat(s)) - ln(1+e^s)], model distribution
MAX_RW = 3968


def _make_stream(block_col_ranges):
    segs, pos = [], 0
    for j, cs, ce in block_col_ranges:
        c = cs
        while c < ce:
            take = min(512 - (pos % 512), ce - c)
            segs.append((j, c, c + take, pos))
            pos += take
            c += take
    return segs, pos


class _Geom:
    def __init__(self, rwp, rwn):
        self.rwp, self.rwn = rwp, rwn
        self.nblk = rwp // 128
        self.posu_segs, self.posu_len = _make_stream(
            [(j, 128 * (j + 1), rwp) for j in range(self.nblk - 1)])
        self.posd_segs, self.posd_len = _make_stream(
            [(j, 128 * j, 128 * (j + 1)) for j in range(self.nblk)])
        self.neg_segs, self.neg_len = _make_stream(
            [(j, 0, rwn) for j in range(self.nblk)])
        self.nu_u = (self.posu_len + UNIT - 1) // UNIT
        self.nu_d = (self.posd_len + UNIT - 1) // UNIT
        self.nu_n = (self.neg_len + UNIT - 1) // UNIT
        self.n_pairs = self.nu_n // 2
        self.has_solo = self.nu_n % 2 == 1
        # widths of each neg unit
        self.nw = [min(UNIT, self.neg_len - UNIT * i) for i in range(self.nu_n)]

        # Ln groups (G=8 folds: 512 cols per pair, solo w>>3), two groups
        np_ = self.n_pairs
        g0_pairs = list(range((np_ + 1) // 2))
        g1_pairs = list(range(len(g0_pairs), np_))
        self.groups = [g0_pairs, g1_pairs]
        self.g_w = [512 * len(g0_pairs),
                    512 * len(g1_pairs) + (256 if self.has_solo else 0)]
        self.pair_group = {}
        for g, ps in enumerate(self.groups):
            for k, p in enumerate(ps):
                self.pair_group[p] = (g, k)

        # choose schraudolph unit count to balance engines (measured ns costs)
        pos_units = [min(UNIT, self.posu_len - UNIT * i) for i in range(self.nu_u)] + \
                    [min(UNIT, self.posd_len - UNIT * i) for i in range(self.nu_d)]
        ln_cols = sum(self.g_w)
        act_fixed = (sum(pos_units) + 312 * len(pos_units)) / 1.2 \
            + 290 * len(pos_units) + (ln_cols + 2 * 312) / 1.2 + 580 + 1900
        dve_fixed = 3320 * self.n_pairs + (2600 if self.has_solo else 0) + 1000
        best = None
        full = [i for i in range(self.nu_n) if self.nw[i] == UNIT and
                not (self.has_solo and i == self.nu_n - 1)]
        for nd in range(0, len(full) + 1):
            act = act_fixed + sum((self.nw[i] + 312) / 1.2 for i in full[nd:]) \
                + (367 if self.has_solo else 0)
            dve = dve_fixed + 2350 * nd
            m = max(act, dve)
            if best is None or m < best[0]:
                best = (m, nd)
        nd = best[1]
        import os
        if os.environ.get("SCHR_ND"):
            nd = int(os.environ["SCHR_ND"])
        # D-units on odd indices (pair-second halves): the following pos
        # unit then reuses a PSUM tile freed by the faster ACT exp instead
        # of waiting on the slower DVE conv
        order = [i for i in full if i % 2 == 1] +                 [i for i in full if i % 2 == 0][::-1]
        self.d_units = frozenset(order[:nd])

        # interleaved unit sequence
        posq = [("U", i) for i in range(self.nu_u)] + \
               [("D", i) for i in range(self.nu_d)]
        seq = [posq[0]] if posq else []
        pi = 1
        for i in range(self.nu_n):
            seq.append(("N", i))
            if i % 2 == 1 and pi < len(posq):
                seq.append(posq[pi]); pi += 1
        seq.extend(posq[pi:])
        self.seq = seq


_geoms = {}


def _get_geom(rwp, rwn):
    key = (rwp, rwn)
    if key not in _geoms:
        _geoms[key] = _Geom(rwp, rwn)
    return _geoms[key]


_compiled = {}


def _build(geom):
    import concourse.bass as bass
    import concourse.mybir as mybir
    import concourse.tile as tile
    from concourse import bacc
    from concourse.hw_specs import get_activation_tables

    def _tables_pref(arch):
        t = get_activation_tables(arch)
        pref = "natural_log_exp_and_others"
        AFt = mybir.ActivationFunctionType
        return {k: (v if k == pref else v - {AFt.Exp, AFt.Ln})
                for k, v in t.items()}

    bacc.get_activation_tables = _tables_pref

    f32 = mybir.dt.float32
    bf16 = mybir.dt.bfloat16
    i16 = mybir.dt.int16
    AF = mybir.ActivationFunctionType
    ALU = mybir.AluOpType
    RWP, RWN = geom.rwp, geom.rwn

    nc = bacc.Bacc("TRN2", target_bir_lowering=False, debug=False,
                   num_devices=N_CORES)
    rp_d = nc.dram_tensor("rp", [64, RWP], bf16, kind="ExternalInput")
    rn_d = nc.dram_tensor("rn", [64, RWN], bf16, kind="ExternalInput")
    accu_d = nc.dram_tensor("accu", [128, geom.nu_u], f32, kind="ExternalOutput")
    accd_d = nc.dram_tensor("accd", [128, geom.nu_d], f32, kind="ExternalOutput")
    accn_d = nc.dram_tensor("accn", [128, 2], f32, kind="ExternalOutput")
    probe_d = nc.dram_tensor("probe", [2], f32, kind="ExternalOutput")

    with tile.TileContext(nc) as tc:
        with (
            tc.tile_pool(name="sb", bufs=1) as sb,
            tc.tile_pool(name="scratch", bufs=2) as scratch_pool,
            tc.tile_pool(name="et2", bufs=3) as et2_pool,
            tc.tile_pool(name="fa", bufs=2) as fa_pool,
            tc.tile_pool(name="fb", bufs=2) as fb_pool,
            tc.tile_pool(name="fc", bufs=2) as fc_pool,
            tc.tile_pool(name="psum", bufs=2, space=bass.MemorySpace.PSUM) as pp,
        ):
            q_sb = sb.tile([128, RWP], bf16, tag="q")
            r_sb = sb.tile([128, RWN], bf16, tag="r")
            nc.sync.dma_start(out=q_sb[0:64, :], in_=rp_d.ap())
            nc.gpsimd.dma_start(out=q_sb[64:128, :], in_=rp_d.ap())
            nc.sync.dma_start(out=r_sb[0:64, :], in_=rn_d.ap())
            nc.gpsimd.dma_start(out=r_sb[64:128, :], in_=rn_d.ap())

            acc_u = sb.tile([128, geom.nu_u], f32, tag="accu")
            acc_d = sb.tile([128, geom.nu_d], f32, tag="accd")
            acc_n = sb.tile([128, 2], f32, tag="accn")
            e0_t = sb.tile([1, 1], f32, tag="e0")
            v0_t = sb.tile([1, 1], f32, tag="v0")
            two_t = sb.tile([1, 1], f32, tag="two")
            nc.vector.memset(two_t[:], 2.0)

            lt0 = sb.tile([128, max(geom.g_w[0], 16)], bf16, tag="lt0")
            lt1 = sb.tile([128, max(geom.g_w[1], 16)], bf16, tag="lt1")
            lt = [lt0, lt1]

            def emit_matmuls(ptile, segs, total, u, rhs_sb, force_lo=None):
                base = u * UNIT
                w = min(UNIT, total - base)
                for (j, c0, c1, pos) in segs:
                    if base <= pos < base + w:
                        lo = (force_lo if force_lo is not None
                              else 64 * ((pos // 512) % 2))
                        nc.tensor.matmul(
                            ptile[:, pos - base:pos - base + (c1 - c0)],
                            q_sb[lo:lo + 64, j * 128:(j + 1) * 128],
                            rhs_sb[lo:lo + 64, c0:c1],
                            start=True, stop=True,
                            tile_position=(lo, 0),
                        )
                return w

            def emit_ln(g):
                ldw = max(max(geom.g_w), 16)
                ld = scratch_pool.tile([128, ldw], bf16, tag="ld")
                wg = geom.g_w[g]
                nc.scalar.activation(ld[:, :wg], lt[g][:, :wg], AF.Ln,
                                     accum_out=acc_n[:, g:g + 1])

            def fold_pair(p, et2t):
                fa = fa_pool.tile([128, 4096], bf16, tag="fa")
                nc.vector.tensor_scalar_add(fa[:], et2t[:], 1.0)
                fb = fb_pool.tile([128, 2048], bf16, tag="fb")
                nc.vector.tensor_tensor(fb[:], fa[:, 0:2048], fa[:, 2048:4096],
                                        op=ALU.mult)
                fc = fc_pool.tile([128, 1024], bf16, tag="fc")
                nc.vector.tensor_tensor(fc[:], fb[:, 0:1024], fb[:, 1024:2048],
                                        op=ALU.mult)
                g, k = geom.pair_group[p]
                nc.vector.tensor_tensor(lt[g][:, 512 * k:512 * (k + 1)],
                                        fc[:, 0:512], fc[:, 512:1024],
                                        op=ALU.mult)

            def fold_solo(ets, w):
                # w-wide unit folded alone into the tail of group 1
                fa = fa_pool.tile([128, 4096], bf16, tag="fa")
                nc.vector.tensor_scalar_add(fa[:, :w], ets[:, :w], 1.0)
                h = w // 2
                fb = fb_pool.tile([128, 2048], bf16, tag="fb")
                nc.vector.tensor_tensor(fb[:, :h], fa[:, 0:h], fa[:, h:w],
                                        op=ALU.mult)
                q = h // 2
                fc = fc_pool.tile([128, 1024], bf16, tag="fc")
                nc.vector.tensor_tensor(fc[:, :q], fb[:, 0:q], fb[:, q:h],
                                        op=ALU.mult)
                o = q // 2
                base = 512 * len(geom.groups[1])
                nc.vector.tensor_tensor(lt[1][:, base:base + o],
                                        fc[:, 0:o], fc[:, o:q], op=ALU.mult)

            et2 = None
            pending_fold = None
            pending_solo = None
            ln_after = {}
            for g in (0, 1):
                # group 1 waits for the solo fold when a solo unit exists
                if geom.groups[g] and not (geom.has_solo and g == 1):
                    ln_after.setdefault(
                        min(geom.groups[g][-1] + 2, geom.n_pairs - 1),
                        []).append(g)

            def flush_fold():
                # folds run one pair late so the DVE FIFO never stalls on a
                # cross-engine (ACT exp) dependency while a conv could run
                nonlocal pending_fold
                if pending_fold is not None:
                    pp_, et2_ = pending_fold
                    fold_pair(pp_, et2_)
                    pending_fold = None
                    if pp_ == ln0_after:
                        emit_ln(0)

            for kind, i in geom.seq:
                ptile = pp.tile([128, UNIT], f32, tag="unit")
                if kind == "U":
                    w = emit_matmuls(ptile, geom.posu_segs, geom.posu_len, i,
                                     q_sb, force_lo=0 if i == 0 else None)
                    st = scratch_pool.tile([128, UNIT], bf16, tag="st")
                    nc.scalar.activation(st[:, :w], ptile[:, :w], AF.Exp,
                                         accum_out=acc_u[:, i:i + 1])
                elif kind == "D":
                    w = emit_matmuls(ptile, geom.posd_segs, geom.posd_len, i, q_sb)
                    st = scratch_pool.tile([128, UNIT], bf16, tag="st")
                    nc.scalar.activation(st[:, :w], ptile[:, :w], AF.Exp,
                                         accum_out=acc_d[:, i:i + 1])
                else:
                    w = emit_matmuls(ptile, geom.neg_segs, geom.neg_len, i,
                                     r_sb, force_lo=0 if i == 0 else None)
                    if geom.has_solo and i == geom.nu_n - 1:
                        ets = scratch_pool.tile([128, UNIT], bf16, tag="ets")
                        nc.scalar.activation(ets[:, :w], ptile[:, :w], AF.Exp)
                        fold_solo(ets, w)
                        emit_ln(1)
                        continue
                    p, half = divmod(i, 2)
                    if half == 0:
                        et2 = et2_pool.tile([128, 4096], bf16, tag="et2")
                    dst = et2[:, 2048 * half:2048 * (half + 1)]
                    if i in geom.d_units:
                        nc.vector.tensor_scalar(
                            out=dst.bitcast(i16), in0=ptile[:, :w],
                            scalar1=SCHR_A, scalar2=SCHR_B,
                            op0=ALU.mult, op1=ALU.add)
                    else:
                        nc.scalar.activation(dst, ptile[:, :w], AF.Exp)
                    if half == 1:
                        fold_pair(p, et2)
                        for g in ln_after.get(p, []):
                            emit_ln(g)

            # table-constant probes: e0 = exp-table(0), v0 = ln-table(2.0)
            nc.scalar.activation(e0_t[:], two_t[:], AF.Exp, scale=0.0)
            nc.scalar.activation(v0_t[:], two_t[:], AF.Ln)

            nc.sync.dma_start(out=accu_d.ap(), in_=acc_u[:])
            nc.sync.dma_start(out=accd_d.ap(), in_=acc_d[:])
            nc.sync.dma_start(out=accn_d.ap(), in_=acc_n[:])
            nc.sync.dma_start(out=probe_d.ap()[0:1], in_=e0_t[:])
            nc.sync.dma_start(out=probe_d.ap()[1:2], in_=v0_t[:])

    nc.compile()
    return nc


def _get_compiled(geom):
    key = (geom.rwp, geom.rwn)
    if key not in _compiled:
        _compiled[key] = _build(geom)
    return _compiled[key]


def _prepare(features, anomaly_prob):
    feat_all = np.asarray(features, dtype=np.float32)[..., 0]
    prob_all = np.asarray(anomaly_prob, dtype=np.float32)[:, 0, :, 0]
    BS, C, N = feat_all.shape
    nns, nas = [], []
    for b in range(BS):
        nn = int((prob_all[b] < np.float32(0.5)).sum())
        nns.append(nn); nas.append(N - nn)
    # floor-cap: the device handles a multiple-of-128 prefix of each side;
    # the few overflow points' pairs are computed exactly on the host below
    rwp = max(128, 128 * (max(nns) // 128))
    rwn = max(128, 128 * (max(nas) // 128))
    if rwp > MAX_RW or rwn > MAX_RW:
        return None, None, None
    geom = _get_geom(rwp, rwn)
    in_maps, metas = [], []
    import ml_dtypes
    for b in range(BS):
        feat, prob = feat_all[b], prob_all[b]
        normal = prob < np.float32(0.5)
        nn, na = nns[b], nas[b]
        nn_dev, na_dev = min(nn, rwp), min(na, rwn)
        norms = np.sqrt(np.sum(feat * feat, axis=0, dtype=np.float32))
        sc = (np.float32(np.sqrt(10.0)) /
              np.maximum(norms, np.float32(1e-12))).astype(np.float32)
        featsc = feat * sc[None, :]
        fn = featsc[:, normal]
        fa = featsc[:, ~normal]
        rp = np.zeros((C, rwp), np.float32)
        rp[:, :nn_dev] = fn[:, :nn_dev]
        rn = np.zeros((C, rwn), np.float32)
        rn[:, :na_dev] = fa[:, :na_dev]
        rp16 = rp.astype(ml_dtypes.bfloat16)
        rn16 = rn.astype(ml_dtypes.bfloat16)
        rpn = rp16[:, :nn_dev].astype(np.float32)
        g = np.zeros(nn_dev, np.float32)
        for c in range(C):
            g = (g + rpn[c] * rpn[c]).astype(np.float32)
        # host-exact contributions of the overflow points (f64):
        #   pos pairs: 2 * (ov_n x main_n) + (ov_n x ov_n offdiag)
        #   neg pairs: (ov_n x all_a) + (main_n x ov_a)
        pos_extra = 0.0
        neg_extra = 0.0

        def _negterm(s):
            sig = 1.0 / (1.0 + np.exp(-s.astype(np.float64)))
            return -np.log(1.0 - sig + 1e-6)

        if nn > nn_dev:
            ov = fn[:, nn_dev:]
            s_om = (ov.T @ fn[:, :nn_dev]).astype(np.float64)
            s_oo = (ov.T @ ov).astype(np.float64)
            pos_extra += 2.0 * float(np.exp(s_om).sum())
            pos_extra += float(np.exp(s_oo).sum()
                               - np.exp(np.diag(s_oo)).sum())
            neg_extra += float(_negterm(ov.T @ fa).sum())
        if na > na_dev:
            ova = fa[:, na_dev:]
            neg_extra += float(_negterm(fn[:, :nn_dev].T @ ova).sum())
        metas.append((nn, na, nn_dev, na_dev, g, pos_extra, neg_extra))
        in_maps.append({"rp": rp16, "rn": rn16})
    return geom, in_maps, metas


def _real_schr_count(geom, nn, na):
    cnt = 0
    for (j, c0, c1, pos) in geom.neg_segs:
        if (pos // UNIT) in geom.d_units:
            nr = min(max(nn - 128 * j, 0), 128)
            cnt += nr * max(0, min(c1, na) - c0)
    return cnt


def _combine(geom, results, metas):
    per_batch, n_valid = [], 0
    NBLK, RWP, RWN = geom.nblk, geom.rwp, geom.rwn
    for r, (nn, na, nn_dev, na_dev, g, pos_extra, neg_extra) in zip(results, metas):
        TA = float(np.asarray(r["accu"], dtype=np.float64).sum())
        TD = float(np.asarray(r["accd"], dtype=np.float64).sum())
        TN = float(np.asarray(r["accn"], dtype=np.float64).sum())
        pr = np.asarray(r["probe"], dtype=np.float64).reshape(-1)
        e0, v0 = pr[0], pr[1]
        S2 = float(np.exp(g.astype(np.float64)).sum())
        nr = np.clip(nn_dev - 128 * np.arange(NBLK), 0, 128)
        nu = np.clip(nn_dev - 128 * (np.arange(NBLK) + 1), 0, None)
        cntU_fake = sum(128 * (RWP - 128 * (j + 1)) - int(nr[j]) * int(nu[j])
                        for j in range(NBLK - 1))
        cntD_fake = sum(128 * 128 - int(nr[j]) * int(nr[j])
                        for j in range(NBLK))
        TA_real = TA - cntU_fake * e0
        TD_real = TD - cntD_fake * e0
        pos_sum = 2.0 * TA_real + TD_real - S2 + pos_extra
        pos_loss = -np.log(pos_sum / max(nn * (nn - 1), 1) + EPS)
        neg_sum = (TN - (RWP * RWN - nn_dev * na_dev) * v0
                   - C_SCHR * _real_schr_count(geom, nn_dev, na_dev)
                   + neg_extra)
        neg_loss = neg_sum / max(nn * na, 1)
        if nn >= 10 and na >= 5:
            n_valid += 1
            per_batch.append(pos_loss + neg_loss)
    total = np.sum(per_batch) / max(n_valid, 1) if per_batch else 0.0
    return np.asarray(total, dtype=np.float32)


def _numpy_fallback(features, anomaly_prob):
    feat_all = np.asarray(features, dtype=np.float32)[..., 0]
    prob_all = np.asarray(anomaly_prob, dtype=np.float32)[:, 0, :, 0]
    BS, C, N = feat_all.shape
    per_batch, n_valid = [], 0
    for b in range(BS):
        feat, prob = feat_all[b], prob_all[b]
        normal = prob < 0.5
        nn = int(normal.sum()); na = N - nn
        norms = np.sqrt(np.sum(feat * feat, axis=0, dtype=np.float32))
        fn = feat / np.maximum(norms, 1e-12)[None, :]
        s = (fn.T @ fn) / np.float32(0.1)
        nm, am = normal, ~normal
        eye = np.eye(N, dtype=bool)
        pm = nm[:, None] & nm[None, :] & ~eye
        pos_mean = np.where(pm, np.exp(s), 0.0).sum() / max(pm.sum(), 1)
        pos_loss = -np.log(pos_mean + EPS)
        cm = nm[:, None] & am[None, :]
        neg = np.where(cm, -np.log(1.0 - 1.0 / (1.0 + np.exp(-s)) + EPS),
                       0.0).sum() / max(cm.sum(), 1)
        if nn >= 10 and na >= 5:
            n_valid += 1
            per_batch.append(pos_loss + neg)
    total = np.sum(per_batch) / max(n_valid, 1) if per_batch else 0.0
    return np.asarray(total, dtype=np.float32)


def kernel(features, anomaly_prob):
    from concourse.bass_utils import run_bass_kernel_spmd
    geom, in_maps, metas = _prepare(features, anomaly_prob)
    if in_maps is None:
        return _numpy_fallback(features, anomaly_prob)
    nc = _get_compiled(geom)
    res = run_bass_kernel_spmd(nc, in_maps, list(range(N_CORES)))
    return _combine(geom, res.results, metas)


# revision 14
# speedup vs baseline: 1.0345x; 1.0345x over previous
"""Trainium2 Bass kernel for the contrastive loss problem — v3.

v3 over v2:
 * Asymmetric padding: the normal side pads to RWP = ceil(max_nn/128)*128
   rows/cols, the anomaly side to RWN = ceil(max_na/128)*128 cols, chosen at
   runtime from the actual inputs (compile is seconds and cached per shape).
 * Input DMAs spread over 4 queues; unit sequence starts and ends with pos
   units; Ln in 2 groups.
 * D-unit (schraudolph) count picked to balance ACT vs DVE busy time.
"""

import numpy as np

UNIT = 2048        # PSUM staging tile width (4 banks); ping-pong 2 tiles = 8 banks
N_CORES = 8
EPS = 1e-6

SCHR_A = 184.66448852539062   # 128/ln2 rounded to fp32
SCHR_B = 16256.0              # 128*127
C_SCHR = 0.02003645           # E[ln(1+e_hat(s)) - ln(1+e^s)], model distribution
MAX_RW = 3968


def _make_stream(block_col_ranges):
    segs, pos = [], 0
    for j, cs, ce in block_col_ranges:
        c = cs
        while c < ce:
            take = min(512 - (pos % 512), ce - c)
            segs.append((j, c, c + take, pos))
            pos += take
            c += take
    return segs, pos


class _Geom:
    def __init__(self, rwp, rwn):
        self.rwp, self.rwn = rwp, rwn
        self.nblk = rwp // 128
        self.posu_segs, self.posu_len = _make_stream(
            [(j, 128 * (j + 1), rwp) for j in range(self.nblk - 1)])
        self.posd_segs, self.posd_len = _make_stream(
            [(j, 128 * j, 128 * (j + 1)) for j in range(self.nblk)])
        self.neg_segs, self.neg_len = _make_stream(
            [(j, 0, rwn) for j in range(self.nblk)])
        self.nu_u = (self.posu_len + UNIT - 1) // UNIT
        self.nu_d = (self.posd_len + UNIT - 1) // UNIT
        self.nu_n = (self.neg_len + UNIT - 1) // UNIT
        self.n_pairs = self.nu_n // 2
        self.has_solo = self.nu_n % 2 == 1
        # widths of each neg unit
        self.nw = [min(UNIT, self.neg_len - UNIT * i) for i in range(self.nu_n)]

        # Ln groups (G=8 folds: 512 cols per pair, solo w>>3), two groups
        np_ = self.n_pairs
        g0_pairs = list(range((np_ + 1) // 2))
        g1_pairs = list(range(len(g0_pairs), np_))
        self.groups = [g0_pairs, g1_pairs]
        self.g_w = [512 * len(g0_pairs),
                    512 * len(g1_pairs) + (256 if self.has_solo else 0)]
        self.pair_group = {}
        for g, ps in enumerate(self.groups):
            for k, p in enumerate(ps):
                self.pair_group[p] = (g, k)

        # choose schraudolph unit count to balance engines (measured ns costs)
        pos_units = [min(UNIT, self.posu_len - UNIT * i) for i in range(self.nu_u)] + \
                    [min(UNIT, self.posd_len - UNIT * i) for i in range(self.nu_d)]
        ln_cols = sum(self.g_w)
        act_fixed = (sum(pos_units) + 312 * len(pos_units)) / 1.2 \
            + 290 * len(pos_units) + (ln_cols + 2 * 312) / 1.2 + 580 + 1900
        dve_fixed = 3320 * self.n_pairs + (2600 if self.has_solo else 0) + 1000
        best = None
        full = [i for i in range(self.nu_n) if self.nw[i] == UNIT and
                not (self.has_solo and i == self.nu_n - 1)]
        for nd in range(0, len(full) + 1):
            act = act_fixed + sum((self.nw[i] + 312) / 1.2 for i in full[nd:]) \
                + (367 if self.has_solo else 0)
            dve = dve_fixed + 2350 * nd
            m = max(act, dve)
            if best is None or m < best[0]:
                best = (m, nd)
        nd = best[1]
        import os
        if os.environ.get("SCHR_ND"):
            nd = int(os.environ["SCHR_ND"])
        # D-units on odd indices (pair-second halves): the following pos
        # unit then reuses a PSUM tile freed by the faster ACT exp instead
        # of waiting on the slower DVE conv
        order = [i for i in full if i % 2 == 1] +                 [i for i in full if i % 2 == 0][::-1]
        self.d_units = frozenset(order[:nd])

        # interleaved unit sequence
        posq = [("U", i) for i in range(self.nu_u)] + \
               [("D", i) for i in range(self.nu_d)]
        seq = [posq[0]] if posq else []
        pi = 1
        for i in range(self.nu_n):
            seq.append(("N", i))
            # pos after every odd neg unit, but the final pos slides one
            # slot earlier so its exp is not queued behind the last Ln
            if pi < len(posq) and (
                    (i % 2 == 1 and i < self.nu_n - 1) or i == self.nu_n - 2):
                seq.append(posq[pi]); pi += 1
        seq.extend(posq[pi:])
        self.seq = seq


_geoms = {}


def _get_geom(rwp, rwn):
    key = (rwp, rwn)
    if key not in _geoms:
        _geoms[key] = _Geom(rwp, rwn)
    return _geoms[key]


_compiled = {}


def _build(geom):
    import concourse.bass as bass
    import concourse.mybir as mybir
    import concourse.tile as tile
    from concourse import bacc
    from concourse.hw_specs import get_activation_tables

    def _tables_pref(arch):
        t = get_activation_tables(arch)
        pref = "natural_log_exp_and_others"
        AFt = mybir.ActivationFunctionType
        return {k: (v if k == pref else v - {AFt.Exp, AFt.Ln})
                for k, v in t.items()}

    bacc.get_activation_tables = _tables_pref

    f32 = mybir.dt.float32
    bf16 = mybir.dt.bfloat16
    i16 = mybir.dt.int16
    AF = mybir.ActivationFunctionType
    ALU = mybir.AluOpType
    RWP, RWN = geom.rwp, geom.rwn

    nc = bacc.Bacc("TRN2", target_bir_lowering=False, debug=False,
                   num_devices=N_CORES)
    rp_d = nc.dram_tensor("rp", [64, RWP], bf16, kind="ExternalInput")
    rn_d = nc.dram_tensor("rn", [64, RWN], bf16, kind="ExternalInput")
    accu_d = nc.dram_tensor("accu", [128, geom.nu_u], f32, kind="ExternalOutput")
    accd_d = nc.dram_tensor("accd", [128, geom.nu_d], f32, kind="ExternalOutput")
    accn_d = nc.dram_tensor("accn", [128, 2], f32, kind="ExternalOutput")
    probe_d = nc.dram_tensor("probe", [2], f32, kind="ExternalOutput")

    with tile.TileContext(nc) as tc:
        with (
            tc.tile_pool(name="sb", bufs=1) as sb,
            tc.tile_pool(name="scratch", bufs=2) as scratch_pool,
            tc.tile_pool(name="et2", bufs=3) as et2_pool,
            tc.tile_pool(name="fa", bufs=2) as fa_pool,
            tc.tile_pool(name="fb", bufs=2) as fb_pool,
            tc.tile_pool(name="fc", bufs=2) as fc_pool,
            tc.tile_pool(name="psum", bufs=2, space=bass.MemorySpace.PSUM) as pp,
        ):
            q_sb = sb.tile([128, RWP], bf16, tag="q")
            r_sb = sb.tile([128, RWN], bf16, tag="r")
            nc.sync.dma_start(out=q_sb[0:64, :], in_=rp_d.ap())
            nc.gpsimd.dma_start(out=q_sb[64:128, :], in_=rp_d.ap())
            nc.sync.dma_start(out=r_sb[0:64, :], in_=rn_d.ap())
            nc.gpsimd.dma_start(out=r_sb[64:128, :], in_=rn_d.ap())

            acc_u = sb.tile([128, geom.nu_u], f32, tag="accu")
            acc_d = sb.tile([128, geom.nu_d], f32, tag="accd")
            acc_n = sb.tile([128, 2], f32, tag="accn")
            e0_t = sb.tile([1, 1], f32, tag="e0")
            v0_t = sb.tile([1, 1], f32, tag="v0")
            two_t = sb.tile([1, 1], f32, tag="two")
            nc.vector.memset(two_t[:], 2.0)

            lt0 = sb.tile([128, max(geom.g_w[0], 16)], bf16, tag="lt0")
            lt1 = sb.tile([128, max(geom.g_w[1], 16)], bf16, tag="lt1")
            lt = [lt0, lt1]

            def emit_matmuls(ptile, segs, total, u, rhs_sb, force_lo=None):
                base = u * UNIT
                w = min(UNIT, total - base)
                for (j, c0, c1, pos) in segs:
                    if base <= pos < base + w:
                        lo = (force_lo if force_lo is not None
                              else 64 * ((pos // 512) % 2))
                        nc.tensor.matmul(
                            ptile[:, pos - base:pos - base + (c1 - c0)],
                            q_sb[lo:lo + 64, j * 128:(j + 1) * 128],
                            rhs_sb[lo:lo + 64, c0:c1],
                            start=True, stop=True,
                            tile_position=(lo, 0),
                        )
                return w

            def emit_ln(g):
                ldw = max(max(geom.g_w), 16)
                ld = scratch_pool.tile([128, ldw], bf16, tag="ld")
                wg = geom.g_w[g]
                nc.scalar.activation(ld[:, :wg], lt[g][:, :wg], AF.Ln,
                                     accum_out=acc_n[:, g:g + 1])

            def fold_pair(p, et2t):
                fa = fa_pool.tile([128, 4096], bf16, tag="fa")
                nc.vector.tensor_scalar_add(fa[:], et2t[:], 1.0)
                fb = fb_pool.tile([128, 2048], bf16, tag="fb")
                nc.vector.tensor_tensor(fb[:], fa[:, 0:2048], fa[:, 2048:4096],
                                        op=ALU.mult)
                fc = fc_pool.tile([128, 1024], bf16, tag="fc")
                nc.vector.tensor_tensor(fc[:], fb[:, 0:1024], fb[:, 1024:2048],
                                        op=ALU.mult)
                g, k = geom.pair_group[p]
                nc.vector.tensor_tensor(lt[g][:, 512 * k:512 * (k + 1)],
                                        fc[:, 0:512], fc[:, 512:1024],
                                        op=ALU.mult)

            def fold_solo(ets, w):
                # w-wide unit folded alone into the tail of group 1
                fa = fa_pool.tile([128, 4096], bf16, tag="fa")
                nc.vector.tensor_scalar_add(fa[:, :w], ets[:, :w], 1.0)
                h = w // 2
                fb = fb_pool.tile([128, 2048], bf16, tag="fb")
                nc.vector.tensor_tensor(fb[:, :h], fa[:, 0:h], fa[:, h:w],
                                        op=ALU.mult)
                q = h // 2
                fc = fc_pool.tile([128, 1024], bf16, tag="fc")
                nc.vector.tensor_tensor(fc[:, :q], fb[:, 0:q], fb[:, q:h],
                                        op=ALU.mult)
                o = q // 2
                base = 512 * len(geom.groups[1])
                nc.vector.tensor_tensor(lt[1][:, base:base + o],
                                        fc[:, 0:o], fc[:, o:q], op=ALU.mult)

            et2 = None
            pending_fold = None
            pending_solo = None
            ln_after = {}
            for g in (0, 1):
                # group 1 waits for the solo fold when a solo unit exists
                if geom.groups[g] and not (geom.has_solo and g == 1):
                    ln_after.setdefault(
                        min(geom.groups[g][-1] + 2, geom.n_pairs - 1),
                        []).append(g)

            def flush_fold():
                # folds run one pair late so the DVE FIFO never stalls on a
                # cross-engine (ACT exp) dependency while a conv could run
                nonlocal pending_fold
                if pending_fold is not None:
                    pp_, et2_ = pending_fold
                    fold_pair(pp_, et2_)
                    pending_fold = None
                    if pp_ == ln0_after:
                        emit_ln(0)

            for kind, i in geom.seq:
                ptile = pp.tile([128, UNIT], f32, tag="unit")
                if kind == "U":
                    w = emit_matmuls(ptile, geom.posu_segs, geom.posu_len, i,
                                     q_sb, force_lo=0 if i == 0 else None)
                    st = scratch_pool.tile([128, UNIT], bf16, tag="st")
                    nc.scalar.activation(st[:, :w], ptile[:, :w], AF.Exp,
                                         accum_out=acc_u[:, i:i + 1])
                elif kind == "D":
                    w = emit_matmuls(ptile, geom.posd_segs, geom.posd_len, i, q_sb)
                    st = scratch_pool.tile([128, UNIT], bf16, tag="st")
                    nc.scalar.activation(st[:, :w], ptile[:, :w], AF.Exp,
                                         accum_out=acc_d[:, i:i + 1])
                else:
                    w = emit_matmuls(ptile, geom.neg_segs, geom.neg_len, i,
                                     r_sb, force_lo=0 if i == 0 else None)
                    if geom.has_solo and i == geom.nu_n - 1:
                        ets = scratch_pool.tile([128, UNIT], bf16, tag="ets")
                        nc.scalar.activation(ets[:, :w], ptile[:, :w], AF.Exp)
                        fold_solo(ets, w)
                        emit_ln(1)
                        continue
                    p, half = divmod(i, 2)
                    if half == 0:
                        et2 = et2_pool.tile([128, 4096], bf16, tag="et2")
                    dst = et2[:, 2048 * half:2048 * (half + 1)]
                    if i in geom.d_units:
                        nc.vector.tensor_scalar(
                            out=dst.bitcast(i16), in0=ptile[:, :w],
                            scalar1=SCHR_A, scalar2=SCHR_B,
                            op0=ALU.mult, op1=ALU.add)
                    else:
                        nc.scalar.activation(dst, ptile[:, :w], AF.Exp)
                    if half == 1:
                        fold_pair(p, et2)
                        for g in ln_after.get(p, []):
                            emit_ln(g)

            # table-constant probes: e0 = exp-table(0), v0 = ln-table(2.0)
            nc.scalar.activation(e0_t[:], two_t[:], AF.Exp, scale=0.0)
            nc.scalar.activation(v0_t[:], two_t[:], AF.Ln)

            nc.sync.dma_start(out=accu_d.ap(), in_=acc_u[:])
            nc.sync.dma_start(out=accd_d.ap(), in_=acc_d[:])
            nc.sync.dma_start(out=accn_d.ap(), in_=acc_n[:])
            nc.sync.dma_start(out=probe_d.ap()[0:1], in_=e0_t[:])
            nc.sync.dma_start(out=probe_d.ap()[1:2], in_=v0_t[:])

    nc.compile()
    return nc


def _get_compiled(geom):
    key = (geom.rwp, geom.rwn)
    if key not in _compiled:
        _compiled[key] = _build(geom)
    return _compiled[key]


def _prepare(features, anomaly_prob):
    feat_all = np.asarray(features, dtype=np.float32)[..., 0]
    prob_all = np.asarray(anomaly_prob, dtype=np.float32)[:, 0, :, 0]
    BS, C, N = feat_all.shape
    nns, nas = [], []
    for b in range(BS):
        nn = int((prob_all[b] < np.float32(0.5)).sum())
        nns.append(nn); nas.append(N - nn)
    # floor-cap: the device handles a multiple-of-128 prefix of each side;
    # the few overflow points' pairs are computed exactly on the host below
    rwp = max(128, 128 * (max(nns) // 128))
    rwn = max(128, 128 * (max(nas) // 128))
    if rwp > MAX_RW or rwn > MAX_RW:
        return None, None, None
    geom = _get_geom(rwp, rwn)
    in_maps, metas = [], []
    import ml_dtypes
    for b in range(BS):
        feat, prob = feat_all[b], prob_all[b]
        normal = prob < np.float32(0.5)
        nn, na = nns[b], nas[b]
        nn_dev, na_dev = min(nn, rwp), min(na, rwn)
        norms = np.sqrt(np.sum(feat * feat, axis=0, dtype=np.float32))
        sc = (np.float32(np.sqrt(10.0)) /
              np.maximum(norms, np.float32(1e-12))).astype(np.float32)
        featsc = feat * sc[None, :]
        fn = featsc[:, normal]
        fa = featsc[:, ~normal]
        rp = np.zeros((C, rwp), np.float32)
        rp[:, :nn_dev] = fn[:, :nn_dev]
        rn = np.zeros((C, rwn), np.float32)
        rn[:, :na_dev] = fa[:, :na_dev]
        rp16 = rp.astype(ml_dtypes.bfloat16)
        rn16 = rn.astype(ml_dtypes.bfloat16)
        rpn = rp16[:, :nn_dev].astype(np.float32)
        g = np.zeros(nn_dev, np.float32)
        for c in range(C):
            g = (g + rpn[c] * rpn[c]).astype(np.float32)
        # host-exact contributions of the overflow points (f64):
        #   pos pairs: 2 * (ov_n x main_n) + (ov_n x ov_n offdiag)
        #   neg pairs: (ov_n x all_a) + (main_n x ov_a)
        pos_extra = 0.0
        neg_extra = 0.0

        def _negterm(s):
            sig = 1.0 / (1.0 + np.exp(-s.astype(np.float64)))
            return -np.log(1.0 - sig + 1e-6)

        if nn > nn_dev:
            ov = fn[:, nn_dev:]
            s_om = (ov.T @ fn[:, :nn_dev]).astype(np.float64)
            s_oo = (ov.T @ ov).astype(np.float64)
            pos_extra += 2.0 * float(np.exp(s_om).sum())
            pos_extra += float(np.exp(s_oo).sum()
                               - np.exp(np.diag(s_oo)).sum())
            neg_extra += float(_negterm(ov.T @ fa).sum())
        if na > na_dev:
            ova = fa[:, na_dev:]
            neg_extra += float(_negterm(fn[:, :nn_dev].T @ ova).sum())
        metas.append((nn, na, nn_dev, na_dev, g, pos_extra, neg_extra))
        in_maps.append({"rp": rp16, "rn": rn16})
    return geom, in_maps, metas


def _real_schr_count(geom, nn, na):
    cnt = 0
    for (j, c0, c1, pos) in geom.neg_segs:
        if (pos // UNIT) in geom.d_units:
            nr = min(max(nn - 128 * j, 0), 128)
            cnt += nr * max(0, min(c1, na) - c0)
    return cnt


def _combine(geom, results, metas):
    per_batch, n_valid = [], 0
    NBLK, RWP, RWN = geom.nblk, geom.rwp, geom.rwn
    for r, (nn, na, nn_dev, na_dev, g, pos_extra, neg_extra) in zip(results, metas):
        TA = float(np.asarray(r["accu"], dtype=np.float64).sum())
        TD = float(np.asarray(r["accd"], dtype=np.float64).sum())
        TN = float(np.asarray(r["accn"], dtype=np.float64).sum())
        pr = np.asarray(r["probe"], dtype=np.float64).reshape(-1)
        e0, v0 = pr[0], pr[1]
        S2 = float(np.exp(g.astype(np.float64)).sum())
        nr = np.clip(nn_dev - 128 * np.arange(NBLK), 0, 128)
        nu = np.clip(nn_dev - 128 * (np.arange(NBLK) + 1), 0, None)
        cntU_fake = sum(128 * (RWP - 128 * (j + 1)) - int(nr[j]) * int(nu[j])
                        for j in range(NBLK - 1))
        cntD_fake = sum(128 * 128 - int(nr[j]) * int(nr[j])
                        for j in range(NBLK))
        TA_real = TA - cntU_fake * e0
        TD_real = TD - cntD_fake * e0
        pos_sum = 2.0 * TA_real + TD_real - S2 + pos_extra
        pos_loss = -np.log(pos_sum / max(nn * (nn - 1), 1) + EPS)
        neg_sum = (TN - (RWP * RWN - nn_dev * na_dev) * v0
                   - C_SCHR * _real_schr_count(geom, nn_dev, na_dev)
                   + neg_extra)
        neg_loss = neg_sum / max(nn * na, 1)
        if nn >= 10 and na >= 5:
            n_valid += 1
            per_batch.append(pos_loss + neg_loss)
    total = np.sum(per_batch) / max(n_valid, 1) if per_batch else 0.0
    return np.asarray(total, dtype=np.float32)


def _numpy_fallback(features, anomaly_prob):
    feat_all = np.asarray(features, dtype=np.float32)[..., 0]
    prob_all = np.asarray(anomaly_prob, dtype=np.float32)[:, 0, :, 0]
    BS, C, N = feat_all.shape
    per_batch, n_valid = [], 0
    for b in range(BS):
        feat, prob = feat_all[b], prob_all[b]
        normal = prob < 0.5
        nn = int(normal.sum()); na = N - nn
        norms = np.sqrt(np.sum(feat * feat, axis=0, dtype=np.float32))
        fn = feat / np.maximum(norms, 1e-12)[None, :]
        s = (fn.T @ fn) / np.float32(0.1)
        nm, am = normal, ~normal
        eye = np.eye(N, dtype=bool)
        pm = nm[:, None] & nm[None, :] & ~eye
        pos_mean = np.where(pm, np.exp(s), 0.0).sum() / max(pm.sum(), 1)
        pos_loss = -np.log(pos_mean + EPS)
        cm = nm[:, None] & am[None, :]
        neg = np.where(cm, -np.log(1.0 - 1.0 / (1.0 + np.exp(-s)) + EPS),
                       0.0).sum() / max(cm.sum(), 1)
        if nn >= 10 and na >= 5:
            n_valid += 1
            per_batch.append(pos_loss + neg)
    total = np.sum(per_batch) / max(n_valid, 1) if per_batch else 0.0
    return np.asarray(total, dtype=np.float32)


def kernel(features, anomaly_prob):
    from concourse.bass_utils import run_bass_kernel_spmd
    geom, in_maps, metas = _prepare(features, anomaly_prob)
    if in_maps is None:
        return _numpy_fallback(features, anomaly_prob)
    nc = _get_compiled(geom)
    res = run_bass_kernel_spmd(nc, in_maps, list(range(N_CORES)))
    return _combine(geom, res.results, metas)
